# revision 12
# baseline (speedup 1.0000x reference)
"""ArcFace loss on 8 TRN2 NeuronCores — class-parallel fp8 (v3).

Per core (C_local=8000, padded to 8192 = 64*128 classes):
  - host passes W-shard x64 as fp8e4 twice: wq[p,j,g,i,c] (DoubleRow
    matmul layout, [d,c]) and wn2[p,t,d] ([c,d] layout for norms)
  - x normalized*512 on device, PE-transposed, cast fp8: xq[g][128,2,512]
  - main matmuls fp8 DoubleRow: 2 per c-tile, contracting 256 of D each
    -> theta_raw^T [128c, 512b] in PSUM
  - class norms: one fused square+row-sum op per c-tile on wn2 tiles
    (DVE scalar_tensor_tensor accum / ScalarE Square accum split),
    nsq -> winv = exp(-0.5*ln(nsq) + ln(S/512)) per-partition; norm
    pipeline runs one chunk ahead of the matmul loop
  - exp(winv_c * theta_raw) on ScalarE -> bf16 ex tiles; class-sum split:
    some tiles via PE ones-matmul PSUM-accumulation, rest via rotating
    DVE bf16 accumulators folded into the same PSUM group at the end
  - target logits: indirect-DMA gather of unquantized W[y] rows + fused
    gpsimd dot/norm (scalar_tensor_tensor accum), exact f32
  - single AllReduce at the end carries [expsum(512) | tgt(512)];
    dummy AllReduces (start + mid) keep the collective path warm
  - final: num = S*(t*cosM - sqrt(1-t^2)*sinM), sqrt via exp/ln,
    loss = -mean(num - log(exp(num) + full_sum - pad - exp(S*t)))
"""

import json
import math

import numpy as np

S = 64.0
MARG = 0.5
EPS = 1e-7
B, D, C = 512, 512, 64000
NCORES = 8
CL = C // NCORES          # 8000
NT = 64                   # c-tiles of 128 per core (padded)
NJ = 16                   # j-blocks of 512 classes
CPAD = NT * 128           # 8192
NPAD = CPAD - CL          # 192 zero-pad classes per core
PAD_ONES = float(NPAD * NCORES)
NCHUNK = 4                # pipeline chunks (16 c-tiles each)
WS = 64.0                 # host W scale into fp8
XS = 512.0                # device xhat scale into fp8
NACC = 4                  # rotating bf16 exp-sum accumulators (DVE side)

_MAX_WAITS = 1


def _split_waits(bir_bytes, max_waits=_MAX_WAITS):
    """walrus in this env rejects >1 sync-wait per instruction; spill extras
    onto preceding wait-only EventSemaphore instructions (same engine)."""
    m = json.loads(bir_bytes)
    uid = [0]
    for f in m.get("functions", []):
        for blk in f.get("blocks", []):
            insts = blk.get("instructions", [])
            out = []
            for i in insts:
                si = i.get("sync_info") or {}
                ws = si.get("on_wait") or []
                if len(ws) > max_waits:
                    keep = ws[-max_waits:]
                    extra = ws[:-max_waits]
                    for cs in range(0, len(extra), max_waits):
                        uid[0] += 1
                        out.append({
                            "name": f"WSPLIT-{uid[0]}",
                            "opcode": "EventSemaphore",
                            "engine": i["engine"],
                            "ins": [],
                            "outs": [],
                            "sync_info": {"on_update": [],
                                          "on_wait": extra[cs:cs + max_waits]},
                        })
                    si["on_wait"] = keep
                out.append(i)
            blk["instructions"] = out
    return json.dumps(m).encode()


def _install_birfix():
    from concourse import bass
    if getattr(bass.Bass, "_birfix_installed", False):
        return
    orig = bass.Bass.to_json_bytes

    def to_json_bytes(self, *a, **k):
        return _split_waits(orig(self, *a, **k))

    bass.Bass.to_json_bytes = to_json_bytes
    bass.Bass._birfix_installed = True


def _norm_on_act(ct):
    # which c-tiles' norm ops run on ScalarE (rest on DVE)
    return ct % 8 == 7


def _sum_on_pe(ct):
    # which c-tiles' exp-sums ride the PE ones-matmul accumulation group
    return ct % 8 < 5


def build(stage=9):
    _install_birfix()
    from concourse import bass, tile, mybir
    from concourse.masks import make_identity

    f32 = mybir.dt.float32
    bf16 = mybir.dt.bfloat16
    f8 = mybir.dt.float8e4
    i32 = mybir.dt.int32
    AX = mybir.AxisListType
    OP = mybir.AluOpType
    AF = mybir.ActivationFunctionType
    DR = mybir.MatmulPerfMode.DoubleRow
    LNS = float(math.log(S / XS))
    LNXS = float(math.log(XS))

    from concourse.tile import add_dep_helper

    nc = bass.Bass("TRN2", target_bir_lowering=False, debug=False,
                   num_devices=NCORES)
    wq = nc.declare_dram_parameter("wq", [128, NJ, 2, 2, 512], f8,
                                   isOutput=False)
    wn2 = nc.declare_dram_parameter("wn2", [128, NT, 512], bf16,
                                    isOutput=False)
    wn = nc.declare_dram_parameter("wn", [CL, D], f32, isOutput=False)
    xx = nc.declare_dram_parameter("x", [B, D], f32, isOutput=False)
    yi = nc.declare_dram_parameter("yi", [128, 4], i32, isOutput=False)
    yv = nc.declare_dram_parameter("yv", [128, 4], f32, isOutput=False)
    out = nc.declare_dram_parameter("out", [1, 1], f32, isOutput=True)

    rg = [list(range(NCORES))]

    last = {}

    def chain(key, inst):
        if key in last:
            add_dep_helper(inst.ins, last[key].ins, False, f"{key} order")
        last[key] = inst
        return inst

    with tile.TileContext(nc) as tc:
        with tc.tile_pool(name="dram", bufs=1, space="DRAM") as dpool, \
             tc.tile_pool(name="const", bufs=1) as cpool, \
             tc.tile_pool(name="big", bufs=1) as big, \
             tc.tile_pool(name="small", bufs=1) as sm, \
             tc.tile_pool(name="scr", bufs=2) as scr, \
             tc.tile_pool(name="expp", bufs=6) as expp, \
             tc.tile_pool(name="gpsum", bufs=2, space="PSUM") as gpsum, \
             tc.tile_pool(name="mpsum", bufs=4, space="PSUM") as mpsum, \
             tc.tile_pool(name="spsum", bufs=1, space="PSUM") as spsum:

            xr = [sm.tile([128, D], f32, name=f"xr{t}") for t in range(4)]
            idx = sm.tile([128, 4], i32, name="idx")
            yvs = sm.tile([128, 4], f32, name="yvs")

            def emit_x_dma():
                for t in range(4):
                    nc.sync.dma_start(out=xr[t][:],
                                      in_=xx[128 * t:128 * (t + 1), :])
                nc.sync.dma_start(out=idx[:], in_=yi[:])
                nc.sync.dma_start(out=yvs[:], in_=yv[:])

            # ---- dummy AllReduce warms the collective path ----
            def emit_dummy_ar(i):
                di = dpool.tile([1, 8], f32, name=f"dmy_i{i}")
                do = dpool.tile([1, 8], f32, name=f"dmy_o{i}",
                                addr_space="Shared")
                chain("gps", nc.gpsimd.collective_compute(
                    "AllReduce", OP.add, replica_groups=rg,
                    ins=[di[:]], outs=[do[:]]))

            with tc.high_priority():
                emit_dummy_ar(0)

            # ---- constants ----
            ident_b = cpool.tile([128, 128], bf16, name="ident_b")
            make_identity(nc, ident_b[:])
            ones_b = cpool.tile([128, 1], bf16, name="ones_b")
            nc.gpsimd.memset(ones_b[:], 1.0)
            ones_f = cpool.tile([128, 1], f32, name="ones_f")
            nc.gpsimd.memset(ones_f[:], 1.0)
            lns_c = cpool.tile([128, 1], f32, name="lns_c")
            nc.gpsimd.memset(lns_c[:], LNS)
            lnxs_c = cpool.tile([128, 1], f32, name="lnxs_c")
            nc.gpsimd.memset(lnxs_c[:], LNXS)

            # ---- W tiles: direct fp8 DMA (chunks) ----
            WQ = big.tile([128, NJ, 2, 2, 512], f8, name="WQ")
            WN2 = big.tile([128, NT, 512], bf16, name="WN2")

            def emit_w_dma(k):
                nc.sync.dma_start(out=WN2[:, 16 * k:16 * (k + 1)],
                                  in_=wn2[:, 16 * k:16 * (k + 1)])
                nc.sync.dma_start(out=WQ[:, 4 * k:4 * (k + 1)],
                                  in_=wq[:, 4 * k:4 * (k + 1)])

            # ---- x path ----
            emit_x_dma()
            emit_w_dma(0)
            emit_w_dma(1)
            xn4 = sm.tile([128, 4], f32, name="xn4")
            for t in range(4):
                sscr = scr.tile([128, D], f32, tag="sscr")
                chain("dve", nc.vector.scalar_tensor_tensor(
                    out=sscr[:], in0=xr[t][:], scalar=1.0, in1=xr[t][:],
                    op0=OP.mult, op1=OP.mult,
                    accum_out=xn4[:, t:t + 1]))
            xn4m = sm.tile([128, 4], f32, name="xn4m")
            chain("dve", nc.vector.tensor_scalar_max(xn4m[:], xn4[:], 1e-30))
            lnx = sm.tile([128, 4], f32, name="lnx")
            chain("act", nc.scalar.activation(out=lnx[:], in_=xn4m[:],
                                              func=AF.Ln))
            # xinv_s = XS / ||x||
            xinv = sm.tile([128, 4], f32, name="xinv")
            chain("act", nc.scalar.activation(out=xinv[:], in_=lnx[:],
                                              func=AF.Exp, scale=-0.5,
                                              bias=lnxs_c[:]))
            xinvp = sm.tile([128, 4], f32, name="xinvp")
            chain("dve", nc.vector.tensor_scalar_mul(xinvp[:], xinv[:],
                                                     1.0 / XS))

            xq = [sm.tile([128, 2, 512], f8, name=f"xq{g}") for g in range(2)]

            def emit_xhat():
                xh = []
                for t in range(4):
                    xht = sm.tile([128, D], bf16, name=f"xh{t}")
                    chain("dve", nc.vector.tensor_scalar_mul(
                        xht[:], xr[t][:], xinv[:, t:t + 1]))
                    xh.append(xht)
                for t in range(4):
                    for k in range(4):
                        tp = gpsum.tile([128, 128], bf16, tag="gp")
                        chain("pe", nc.tensor.transpose(
                            tp[:], xh[t][:, 128 * k:128 * (k + 1)],
                            ident_b[:]))
                        chain("dve", nc.vector.tensor_copy(
                            xq[k // 2][:, k % 2, 128 * t:128 * (t + 1)],
                            tp[:]))

            if stage == 1:
                emit_xhat()
                probe = sm.tile([1, 1], f32, name="probe")
                nc.vector.tensor_copy(probe[:], xinv[0:1, 0:1])
                nc.sync.dma_start(out=out[:], in_=probe[:])
                return nc

            # ---- gather path: W[y] rows (f32 exact), fused on gpsimd ----
            tgt = sm.tile([128, 4], f32, name="tgt")
            wsel = sm.tile([128, 4, D], f32, name="wsel")

            def emit_gather_dma():
                for t in range(4):
                    chain("gps", nc.gpsimd.indirect_dma_start(
                        out=wsel[:, t, :], out_offset=None, in_=wn[:],
                        in_offset=bass.IndirectOffsetOnAxis(
                            ap=idx[:, t:t + 1], axis=0)))

            dots = sm.tile([128, 4], f32, name="dots")
            wsq = sm.tile([128, 4], f32, name="wsq")

            def emit_gather_piece(t):
                gtr = scr.tile([128, D], f32, tag="gtr", bufs=2)
                chain("dve", nc.vector.scalar_tensor_tensor(
                    out=gtr[:], in0=wsel[:, t, :], scalar=1.0,
                    in1=xr[t][:], op0=OP.mult, op1=OP.mult,
                    accum_out=dots[:, t:t + 1]))
                gtr2 = scr.tile([128, D], f32, tag="gtr", bufs=2)
                chain("dve", nc.vector.scalar_tensor_tensor(
                    out=gtr2[:], in0=wsel[:, t, :], scalar=1.0,
                    in1=wsel[:, t, :], op0=OP.mult, op1=OP.mult,
                    accum_out=wsq[:, t:t + 1]))

            def emit_gather_compute():
                wsqm = sm.tile([128, 4], f32, name="wsqm")
                chain("dve", nc.vector.tensor_scalar_max(wsqm[:], wsq[:],
                                                         1e-30))
                lnw = sm.tile([128, 4], f32, name="lnw")
                chain("act", nc.scalar.activation(out=lnw[:], in_=wsqm[:],
                                                  func=AF.Ln))
                wsinv = sm.tile([128, 4], f32, name="wsinv")
                chain("act", nc.scalar.activation(out=wsinv[:], in_=lnw[:],
                                                  func=AF.Exp, scale=-0.5))
                tg0 = sm.tile([128, 4], f32, name="tg0")
                chain("dve", nc.vector.tensor_tensor(tg0[:], dots[:],
                                                     xinvp[:], OP.mult))
                tg1 = sm.tile([128, 4], f32, name="tg1")
                chain("dve", nc.vector.tensor_tensor(tg1[:], tg0[:],
                                                     wsinv[:], OP.mult))
                chain("dve", nc.vector.tensor_tensor(tgt[:], tg1[:], yvs[:],
                                                     OP.mult))

            # ---- norms: fused square+row-sum per c-tile (DVE/ACT split) ----
            nsqc = sm.tile([128, NT], f32, name="nsqc")
            winv = sm.tile([128, NT], f32, name="winv")
            trD = sm.tile([128, 512], bf16, name="trD")
            trA = sm.tile([128, 512], bf16, name="trA")

            def emit_norm(ct):
                if _norm_on_act(ct):
                    chain("act", nc.scalar.activation(
                        out=trA[:], in_=WN2[:, ct], func=AF.Square,
                        accum_out=nsqc[:, ct:ct + 1]))
                else:
                    chain("dve", nc.vector.scalar_tensor_tensor(
                        out=trD[:], in0=WN2[:, ct], scalar=1.0,
                        in1=WN2[:, ct], op0=OP.mult, op1=OP.mult,
                        accum_out=nsqc[:, ct:ct + 1]))

            def emit_winv(c0, c1):
                n = c1 - c0
                nsqm = scr.tile([128, 16], f32, tag="nsqm", bufs=4)
                chain("dve", nc.vector.tensor_scalar_max(
                    nsqm[:, 0:n], nsqc[:, c0:c1], 1e-30))
                lnn = scr.tile([128, 16], f32, tag="lnn", bufs=4)
                chain("act", nc.scalar.activation(out=lnn[:, 0:n],
                                                  in_=nsqm[:, 0:n],
                                                  func=AF.Ln))
                chain("act", nc.scalar.activation(
                    out=winv[:, c0:c1], in_=lnn[:, 0:n], func=AF.Exp,
                    scale=-0.5, bias=lns_c[:]))

            def emit_winv_chunk(k):
                emit_winv(16 * k, 16 * (k + 1))

            if stage == 3:
                for k in range(NCHUNK):
                    if k < NCHUNK - 1:
                        emit_w_dma(k + 1)
                    for ct in range(16 * k, 16 * (k + 1)):
                        emit_norm(ct)
                    emit_winv_chunk(k)
                probe = sm.tile([1, 1], f32, name="probe")
                nc.vector.tensor_copy(probe[:], winv[0:1, 0:1])
                nc.sync.dma_start(out=out[:], in_=probe[:])
                return nc

            # ---- main loop (norm pipeline runs one chunk ahead) ----
            emit_xhat()
            accs = [sm.tile([128, B], bf16, name=f"acc{a}")
                    for a in range(NACC)]
            sump = spsum.tile([1, B], f32, tag="sp", name="sump")
            sum_state = {"started": False, "ndve": 0}

            for g4 in range(4):
                for ct in range(4 * g4, 4 * g4 + 4):
                    emit_norm(ct)
                emit_winv(4 * g4, 4 * g4 + 4)

            pend = None  # (ex_tile, ct)

            def emit_sum(ex, ct):
                if _sum_on_pe(ct):
                    chain("pe", nc.tensor.matmul(
                        sump[:], lhsT=ones_b[:], rhs=ex[:],
                        start=not sum_state["started"], stop=False,
                        skip_group_check=True))
                    sum_state["started"] = True
                else:
                    a = sum_state["ndve"] % NACC
                    if sum_state["ndve"] < NACC:
                        chain("dve", nc.vector.tensor_copy(accs[a][:], ex[:]))
                    else:
                        chain("dve", nc.vector.tensor_tensor(
                            accs[a][:], accs[a][:], ex[:], OP.add))
                    sum_state["ndve"] += 1

            for k in range(NCHUNK):
                if k < NCHUNK - 2:
                    emit_w_dma(k + 2)
                if k == 1:
                    emit_gather_dma()
                for ct in range(16 * k, 16 * (k + 1)):
                    j, s = ct // 4, ct % 4
                    mp = mpsum.tile([128, B], f32, tag="mp")
                    for g in range(2):
                        chain("pe", nc.tensor.matmul(
                            mp[:],
                            lhsT=WQ[:, j, g, :, 128 * s:128 * (s + 1)],
                            rhs=xq[g][:], start=(g == 0), stop=(g == 1),
                            perf_mode=DR))
                    ex = expp.tile([128, B], bf16, tag="ex")
                    chain("act", nc.scalar.activation(
                        out=ex[:], in_=mp[:], func=AF.Exp,
                        scale=winv[:, ct:ct + 1]))
                    if pend is not None:
                        emit_sum(*pend)
                    pend = (ex, ct)
                    if 32 <= ct < 48 and ct % 4 == 1:
                        emit_gather_piece((ct - 33) // 4)
                    if ct == 46:
                        emit_gather_compute()
                    # next chunk's norm pipeline, spread one-or-two ops
                    # per tile so the DVE queue never blocks on a burst
                    ti = ct % 16
                    if k < NCHUNK - 1 and ti >= 2:
                        base = 16 * (k + 1)
                        if ti <= 13:
                            emit_norm(base + ti - 2)
                        elif ti == 14:
                            emit_norm(base + 12)
                            emit_norm(base + 13)
                        else:
                            emit_norm(base + 14)
                            emit_norm(base + 15)
                            emit_winv_chunk(k + 1)
            emit_sum(*pend)

            # ---- fold DVE accumulators into the PE PSUM group ----
            nacc = min(NACC, sum_state["ndve"])
            stride = 1
            while stride < nacc:
                for a in range(0, nacc - stride, 2 * stride):
                    chain("dve", nc.vector.tensor_tensor(
                        accs[a][:], accs[a][:], accs[a + stride][:], OP.add))
                stride *= 2
            chain("pe", nc.tensor.matmul(
                sump[:], lhsT=ones_b[:], rhs=accs[0][:],
                start=not sum_state["started"], stop=True,
                skip_group_check=True))
            sumrow = sm.tile([1, B], f32, name="sumrow")
            chain("act", nc.scalar.activation(out=sumrow[:], in_=sump[:],
                                              func=AF.Copy))
            if stage == 4:
                probe = sm.tile([1, 1], f32, name="probe")
                nc.vector.tensor_copy(probe[:], sumrow[0:1, 0:1])
                nc.sync.dma_start(out=out[:], in_=probe[:])
                return nc

            # ---- single AllReduce: [sums | tgt] (staging on HW DGE) ----
            arin = dpool.tile([1, 2 * B], f32, name="arin")
            arout = dpool.tile([1, 2 * B], f32, name="arout",
                               addr_space="Shared")
            nc.sync.dma_start(out=arin[0:1, 0:B], in_=sumrow[:])
            nc.sync.dma_start(
                out=arin[0:1, B:2 * B].rearrange("a (j p) -> (a p) j", p=128),
                in_=tgt[:])
            chain("gps", nc.gpsimd.collective_compute(
                "AllReduce", OP.add, replica_groups=rg,
                ins=[arin[:]], outs=[arout[:]]))

            fsa = sm.tile([128, 4], f32, name="fsa")
            tg = sm.tile([128, 4], f32, name="tg")
            nc.sync.dma_start(
                out=fsa[:],
                in_=arout[0:1, 0:B].rearrange("a (j p) -> (a p) j", p=128))
            nc.sync.dma_start(
                out=tg[:],
                in_=arout[0:1, B:2 * B].rearrange("a (j p) -> (a p) j",
                                                  p=128))

            # ---- final phase ----
            tcl = sm.tile([128, 4], f32, name="tcl")
            chain("dve", nc.vector.tensor_scalar(
                tcl[:], tg[:], -1.0 + EPS, 1.0 - EPS, OP.max, OP.min))
            om = sm.tile([128, 4], f32, name="om")
            # om = 1 - tcl^2 = (tcl * -1) * tcl + 1... use two ops:
            t2 = sm.tile([128, 4], f32, name="t2")
            chain("dve", nc.vector.tensor_tensor(t2[:], tcl[:], tcl[:],
                                                 OP.mult))
            chain("dve", nc.vector.tensor_scalar(om[:], t2[:], -1.0, 1.0,
                                                 OP.mult, OP.add))
            lnom = sm.tile([128, 4], f32, name="lnom")
            chain("act", nc.scalar.activation(out=lnom[:], in_=om[:],
                                              func=AF.Ln))
            # root' = S*sin(M)*sqrt(om) = exp(0.5*lnom + ln(S*sinM))
            lnsm_c = cpool.tile([128, 1], f32, name="lnsm_c")
            nc.gpsimd.memset(lnsm_c[:], float(math.log(S * math.sin(MARG))))
            rootp = sm.tile([128, 4], f32, name="rootp")
            chain("act", nc.scalar.activation(out=rootp[:], in_=lnom[:],
                                              func=AF.Exp, scale=0.5,
                                              bias=lnsm_c[:]))
            # num = tcl*S*cosM - root'
            num = sm.tile([128, 4], f32, name="num")
            chain("dve", nc.vector.scalar_tensor_tensor(
                out=num[:], in0=tcl[:], scalar=float(S * math.cos(MARG)),
                in1=rootp[:], op0=OP.mult, op1=OP.subtract))
            expnum = sm.tile([128, 4], f32, name="expnum")
            chain("act", nc.scalar.activation(out=expnum[:], in_=num[:],
                                              func=AF.Exp))
            est = sm.tile([128, 4], f32, name="est")
            chain("act", nc.scalar.activation(out=est[:], in_=tg[:],
                                              func=AF.Exp, scale=S))
            # d2 = (fsa - PAD_ONES) - est ; den = d2 + expnum
            d2 = sm.tile([128, 4], f32, name="d2")
            chain("dve", nc.vector.scalar_tensor_tensor(
                out=d2[:], in0=fsa[:], scalar=-PAD_ONES, in1=est[:],
                op0=OP.add, op1=OP.subtract))
            den = sm.tile([128, 4], f32, name="den")
            chain("dve", nc.vector.tensor_tensor(den[:], d2[:], expnum[:],
                                                 OP.add))
            lden = sm.tile([128, 4], f32, name="lden")
            chain("act", nc.scalar.activation(out=lden[:], in_=den[:],
                                              func=AF.Ln))
            # pb = num - lden ; pr = rowsum(pb)
            pbt = sm.tile([128, 4], f32, name="pbt")
            pr = sm.tile([128, 1], f32, name="pr")
            chain("dve", nc.vector.tensor_tensor(pbt[:], num[:], lden[:],
                                                 OP.subtract))
            chain("dve", nc.vector.tensor_reduce(out=pr[:], in_=pbt[:],
                                                 axis=AX.X, op=OP.add))
            fmm = spsum.tile([1, 1], f32, tag="sp2", name="fmm")
            nc.tensor.matmul(fmm[:], lhsT=ones_f[:], rhs=pr[:], start=True,
                             stop=True)
            outsb = sm.tile([1, 1], f32, name="outsb")
            nc.scalar.activation(out=outsb[:], in_=fmm[:], func=AF.Copy,
                                 scale=-1.0 / B)
            nc.sync.dma_start(out=out[:], in_=outsb[:])

    return nc


_CACHE = {}


def make_in_maps(x, y, W):
    import ml_dtypes
    x = np.ascontiguousarray(np.asarray(x, dtype=np.float32))
    y = np.asarray(y).astype(np.int64)
    W = np.asarray(W, dtype=np.float32)
    in_maps = []
    for i in range(NCORES):
        c0 = i * CL
        Wsh = np.ascontiguousarray(W[c0:c0 + CL])           # [CL, D]
        Wp = np.zeros((CPAD, D), dtype=np.float32)
        Wp[:CL] = Wsh * WS
        # wq[p, j, g, i, c] = WS * W^T[g*256+i*128+p, j*512+c]
        wq_i = np.ascontiguousarray(
            Wp.reshape(NJ, 512, 2, 2, 128).transpose(4, 0, 2, 3, 1)
        ).astype(ml_dtypes.float8_e4m3)
        # wn2[p, t, d] = WS * W[t*128+p, d]
        wn2_i = np.ascontiguousarray(
            Wp.reshape(NT, 128, D).transpose(1, 0, 2)
        ).astype(ml_dtypes.bfloat16)
        yloc = np.clip(y - c0, 0, CL - 1).astype(np.int32)  # [B]
        valid = ((y >= c0) & (y < c0 + CL)).astype(np.float32)
        in_maps.append({
            "wq": wq_i,
            "wn2": wn2_i,
            "wn": Wsh,
            "x": x,
            "yi": np.ascontiguousarray(yloc.reshape(4, 128).T),
            "yv": np.ascontiguousarray(valid.reshape(4, 128).T),
        })
    return in_maps


def kernel(x, y, W, _trace=False, _stage=9):
    from concourse.bass_utils import run_bass_kernel_spmd
    key = ("nc", _stage)
    if key not in _CACHE:
        _CACHE[key] = build(_stage)
    in_maps = make_in_maps(x, y, W)
    res = run_bass_kernel_spmd(_CACHE[key], in_maps, list(range(NCORES)),
                               trace=_trace)
    val = np.float32(res.results[0]["out"][0, 0])
    if _trace:
        return val, res
    return val


# revision 13
# speedup vs baseline: 1.0267x; 1.0267x over previous
"""ArcFace loss on 8 TRN2 NeuronCores — class-parallel fp8 (v3).

Per core (C_local=8000, padded to 8192 = 64*128 classes):
  - host passes W-shard x64 as fp8e4 twice: wq[p,j,g,i,c] (DoubleRow
    matmul layout, [d,c]) and wn2[p,t,d] ([c,d] layout for norms)
  - x normalized*512 on device, PE-transposed, cast fp8: xq[g][128,2,512]
  - main matmuls fp8 DoubleRow: 2 per c-tile, contracting 256 of D each
    -> theta_raw^T [128c, 512b] in PSUM
  - class norms: one fused square+row-sum op per c-tile on wn2 tiles
    (DVE scalar_tensor_tensor accum / ScalarE Square accum split),
    nsq -> winv = exp(-0.5*ln(nsq) + ln(S/512)) per-partition; norm
    pipeline runs one chunk ahead of the matmul loop
  - exp(winv_c * theta_raw) on ScalarE -> bf16 ex tiles; class-sum split:
    some tiles via PE ones-matmul PSUM-accumulation, rest via rotating
    DVE bf16 accumulators folded into the same PSUM group at the end
  - target logits: indirect-DMA gather of unquantized W[y] rows + fused
    gpsimd dot/norm (scalar_tensor_tensor accum), exact f32
  - single AllReduce at the end carries [expsum(512) | tgt(512)];
    dummy AllReduces (start + mid) keep the collective path warm
  - final: num = S*(t*cosM - sqrt(1-t^2)*sinM), sqrt via exp/ln,
    loss = -mean(num - log(exp(num) + full_sum - pad - exp(S*t)))
"""

import json
import math

import numpy as np

S = 64.0
MARG = 0.5
EPS = 1e-7
B, D, C = 512, 512, 64000
NCORES = 8
CL = C // NCORES          # 8000
NT = 64                   # c-tiles of 128 per core (padded)
NJ = 16                   # j-blocks of 512 classes
CPAD = NT * 128           # 8192
NPAD = CPAD - CL          # 192 zero-pad classes per core
PAD_ONES = float(NPAD * NCORES)
NCHUNK = 4                # pipeline chunks (16 c-tiles each)
WS = 64.0                 # host W scale into fp8
XS = 512.0                # device xhat scale into fp8
NACC = 4                  # rotating bf16 exp-sum accumulators (DVE side)

_MAX_WAITS = 1


def _split_waits(bir_bytes, max_waits=_MAX_WAITS):
    """walrus in this env rejects >1 sync-wait per instruction; spill extras
    onto preceding wait-only EventSemaphore instructions (same engine)."""
    m = json.loads(bir_bytes)
    uid = [0]
    for f in m.get("functions", []):
        for blk in f.get("blocks", []):
            insts = blk.get("instructions", [])
            out = []
            for i in insts:
                si = i.get("sync_info") or {}
                ws = si.get("on_wait") or []
                if len(ws) > max_waits:
                    keep = ws[-max_waits:]
                    extra = ws[:-max_waits]
                    for cs in range(0, len(extra), max_waits):
                        uid[0] += 1
                        out.append({
                            "name": f"WSPLIT-{uid[0]}",
                            "opcode": "EventSemaphore",
                            "engine": i["engine"],
                            "ins": [],
                            "outs": [],
                            "sync_info": {"on_update": [],
                                          "on_wait": extra[cs:cs + max_waits]},
                        })
                    si["on_wait"] = keep
                out.append(i)
            blk["instructions"] = out
    return json.dumps(m).encode()


def _install_birfix():
    from concourse import bass
    if getattr(bass.Bass, "_birfix_installed", False):
        return
    orig = bass.Bass.to_json_bytes

    def to_json_bytes(self, *a, **k):
        return _split_waits(orig(self, *a, **k))

    bass.Bass.to_json_bytes = to_json_bytes
    bass.Bass._birfix_installed = True


def _norm_on_act(ct):
    # which c-tiles' norm ops run on ScalarE (rest on DVE)
    return False


def _sum_on_pe(ct):
    # which c-tiles' exp-sums ride the PE ones-matmul accumulation group
    return ct % 2 == 0


def build(stage=9):
    _install_birfix()
    from concourse import bass, tile, mybir
    from concourse.masks import make_identity

    f32 = mybir.dt.float32
    bf16 = mybir.dt.bfloat16
    f8 = mybir.dt.float8e4
    i32 = mybir.dt.int32
    AX = mybir.AxisListType
    OP = mybir.AluOpType
    AF = mybir.ActivationFunctionType
    DR = mybir.MatmulPerfMode.DoubleRow
    LNS = float(math.log(S / XS))
    LNXS = float(math.log(XS))

    from concourse.tile import add_dep_helper

    nc = bass.Bass("TRN2", target_bir_lowering=False, debug=False,
                   num_devices=NCORES)
    wq = nc.declare_dram_parameter("wq", [128, NJ, 2, 2, 512], f8,
                                   isOutput=False)
    wn2 = nc.declare_dram_parameter("wn2", [128, NT, 512], bf16,
                                    isOutput=False)
    wn = nc.declare_dram_parameter("wn", [CL, D], f32, isOutput=False)
    xx = nc.declare_dram_parameter("x", [B, D], f32, isOutput=False)
    yi = nc.declare_dram_parameter("yi", [128, 4], i32, isOutput=False)
    yv = nc.declare_dram_parameter("yv", [128, 4], f32, isOutput=False)
    out = nc.declare_dram_parameter("out", [1, 1], f32, isOutput=True)

    rg = [list(range(NCORES))]

    last = {}

    def chain(key, inst):
        if key in last:
            add_dep_helper(inst.ins, last[key].ins, False, f"{key} order")
        last[key] = inst
        return inst

    with tile.TileContext(nc) as tc:
        with tc.tile_pool(name="dram", bufs=1, space="DRAM") as dpool, \
             tc.tile_pool(name="const", bufs=1) as cpool, \
             tc.tile_pool(name="big", bufs=1) as big, \
             tc.tile_pool(name="small", bufs=1) as sm, \
             tc.tile_pool(name="scr", bufs=2) as scr, \
             tc.tile_pool(name="expp", bufs=6) as expp, \
             tc.tile_pool(name="gpsum", bufs=2, space="PSUM") as gpsum, \
             tc.tile_pool(name="mpsum", bufs=4, space="PSUM") as mpsum, \
             tc.tile_pool(name="spsum", bufs=1, space="PSUM") as spsum:

            xr = [sm.tile([128, D], f32, name=f"xr{t}") for t in range(4)]
            idx = sm.tile([128, 4], i32, name="idx")
            yvs = sm.tile([128, 4], f32, name="yvs")

            def emit_x_dma():
                for t in range(4):
                    nc.sync.dma_start(out=xr[t][:],
                                      in_=xx[128 * t:128 * (t + 1), :])
                nc.sync.dma_start(out=idx[:], in_=yi[:])
                nc.sync.dma_start(out=yvs[:], in_=yv[:])

            # ---- dummy AllReduce warms the collective path ----
            def emit_dummy_ar(i):
                di = dpool.tile([1, 8], f32, name=f"dmy_i{i}")
                do = dpool.tile([1, 8], f32, name=f"dmy_o{i}",
                                addr_space="Shared")
                chain("gps", nc.gpsimd.collective_compute(
                    "AllReduce", OP.add, replica_groups=rg,
                    ins=[di[:]], outs=[do[:]]))

            with tc.high_priority():
                emit_dummy_ar(0)

            # ---- constants ----
            ident_b = cpool.tile([128, 128], bf16, name="ident_b")
            make_identity(nc, ident_b[:])
            ones_b = cpool.tile([128, 1], bf16, name="ones_b")
            nc.gpsimd.memset(ones_b[:], 1.0)
            ones_f = cpool.tile([128, 1], f32, name="ones_f")
            nc.gpsimd.memset(ones_f[:], 1.0)
            lns_c = cpool.tile([128, 1], f32, name="lns_c")
            nc.gpsimd.memset(lns_c[:], LNS)
            lnxs_c = cpool.tile([128, 1], f32, name="lnxs_c")
            nc.gpsimd.memset(lnxs_c[:], LNXS)

            # ---- W tiles: direct fp8 DMA (chunks) ----
            WQ = big.tile([128, NJ, 2, 2, 512], f8, name="WQ")
            WN2 = big.tile([128, NT, 512], bf16, name="WN2")

            def emit_w_dma(k):
                nc.sync.dma_start(out=WN2[:, 16 * k:16 * (k + 1)],
                                  in_=wn2[:, 16 * k:16 * (k + 1)])
                nc.sync.dma_start(out=WQ[:, 4 * k:4 * (k + 1)],
                                  in_=wq[:, 4 * k:4 * (k + 1)])

            # ---- x path ----
            emit_x_dma()
            emit_w_dma(0)
            emit_w_dma(1)
            xn4 = sm.tile([128, 4], f32, name="xn4")
            for t in range(4):
                sscr = scr.tile([128, D], f32, tag="sscr")
                chain("dve", nc.vector.scalar_tensor_tensor(
                    out=sscr[:], in0=xr[t][:], scalar=1.0, in1=xr[t][:],
                    op0=OP.mult, op1=OP.mult,
                    accum_out=xn4[:, t:t + 1]))
            xn4m = sm.tile([128, 4], f32, name="xn4m")
            chain("dve", nc.vector.tensor_scalar_max(xn4m[:], xn4[:], 1e-30))
            lnx = sm.tile([128, 4], f32, name="lnx")
            chain("act", nc.scalar.activation(out=lnx[:], in_=xn4m[:],
                                              func=AF.Ln))
            # xinv_s = XS / ||x||
            xinv = sm.tile([128, 4], f32, name="xinv")
            chain("act", nc.scalar.activation(out=xinv[:], in_=lnx[:],
                                              func=AF.Exp, scale=-0.5,
                                              bias=lnxs_c[:]))
            xinvp = sm.tile([128, 4], f32, name="xinvp")
            chain("dve", nc.vector.tensor_scalar_mul(xinvp[:], xinv[:],
                                                     1.0 / XS))

            xq = [sm.tile([128, 2, 512], f8, name=f"xq{g}") for g in range(2)]

            def emit_xhat():
                xh = []
                for t in range(4):
                    xht = sm.tile([128, D], bf16, name=f"xh{t}")
                    chain("dve", nc.vector.tensor_scalar_mul(
                        xht[:], xr[t][:], xinv[:, t:t + 1]))
                    xh.append(xht)
                for t in range(4):
                    for k in range(4):
                        tp = gpsum.tile([128, 128], bf16, tag="gp")
                        chain("pe", nc.tensor.transpose(
                            tp[:], xh[t][:, 128 * k:128 * (k + 1)],
                            ident_b[:]))
                        chain("dve", nc.vector.tensor_copy(
                            xq[k // 2][:, k % 2, 128 * t:128 * (t + 1)],
                            tp[:]))

            if stage == 1:
                emit_xhat()
                probe = sm.tile([1, 1], f32, name="probe")
                nc.vector.tensor_copy(probe[:], xinv[0:1, 0:1])
                nc.sync.dma_start(out=out[:], in_=probe[:])
                return nc

            # ---- gather path: W[y] rows (f32 exact), fused on gpsimd ----
            tgt = sm.tile([128, 4], f32, name="tgt")
            wsel = sm.tile([128, 4, D], f32, name="wsel")

            def emit_gather_dma():
                for t in range(4):
                    chain("gps", nc.gpsimd.indirect_dma_start(
                        out=wsel[:, t, :], out_offset=None, in_=wn[:],
                        in_offset=bass.IndirectOffsetOnAxis(
                            ap=idx[:, t:t + 1], axis=0)))

            dots = sm.tile([128, 4], f32, name="dots")
            wsq = sm.tile([128, 4], f32, name="wsq")

            def emit_gather_piece(t):
                gtr = scr.tile([128, D], f32, tag="gtr", bufs=2)
                chain("dve", nc.vector.scalar_tensor_tensor(
                    out=gtr[:], in0=wsel[:, t, :], scalar=1.0,
                    in1=xr[t][:], op0=OP.mult, op1=OP.mult,
                    accum_out=dots[:, t:t + 1]))
                gtr2 = scr.tile([128, D], f32, tag="gtr", bufs=2)
                chain("dve", nc.vector.scalar_tensor_tensor(
                    out=gtr2[:], in0=wsel[:, t, :], scalar=1.0,
                    in1=wsel[:, t, :], op0=OP.mult, op1=OP.mult,
                    accum_out=wsq[:, t:t + 1]))

            def emit_gather_compute():
                wsqm = sm.tile([128, 4], f32, name="wsqm")
                chain("dve", nc.vector.tensor_scalar_max(wsqm[:], wsq[:],
                                                         1e-30))
                lnw = sm.tile([128, 4], f32, name="lnw")
                chain("act", nc.scalar.activation(out=lnw[:], in_=wsqm[:],
                                                  func=AF.Ln))
                wsinv = sm.tile([128, 4], f32, name="wsinv")
                chain("act", nc.scalar.activation(out=wsinv[:], in_=lnw[:],
                                                  func=AF.Exp, scale=-0.5))
                tg0 = sm.tile([128, 4], f32, name="tg0")
                chain("dve", nc.vector.tensor_tensor(tg0[:], dots[:],
                                                     xinvp[:], OP.mult))
                tg1 = sm.tile([128, 4], f32, name="tg1")
                chain("dve", nc.vector.tensor_tensor(tg1[:], tg0[:],
                                                     wsinv[:], OP.mult))
                chain("dve", nc.vector.tensor_tensor(tgt[:], tg1[:], yvs[:],
                                                     OP.mult))

            # ---- norms: fused square+row-sum per c-tile (DVE/ACT split) ----
            nsqc = sm.tile([128, NT], f32, name="nsqc")
            winv = sm.tile([128, NT], f32, name="winv")
            trD = sm.tile([128, 512], bf16, name="trD")
            trA = sm.tile([128, 512], bf16, name="trA")

            def emit_norm(ct):
                if _norm_on_act(ct):
                    chain("act", nc.scalar.activation(
                        out=trA[:], in_=WN2[:, ct], func=AF.Square,
                        accum_out=nsqc[:, ct:ct + 1]))
                else:
                    chain("dve", nc.vector.scalar_tensor_tensor(
                        out=trD[:], in0=WN2[:, ct], scalar=1.0,
                        in1=WN2[:, ct], op0=OP.mult, op1=OP.mult,
                        accum_out=nsqc[:, ct:ct + 1]))

            def emit_winv(c0, c1):
                n = c1 - c0
                nsqm = scr.tile([128, 16], f32, tag="nsqm", bufs=4)
                chain("dve", nc.vector.tensor_scalar_max(
                    nsqm[:, 0:n], nsqc[:, c0:c1], 1e-30))
                lnn = scr.tile([128, 16], f32, tag="lnn", bufs=4)
                chain("act", nc.scalar.activation(out=lnn[:, 0:n],
                                                  in_=nsqm[:, 0:n],
                                                  func=AF.Ln))
                chain("act", nc.scalar.activation(
                    out=winv[:, c0:c1], in_=lnn[:, 0:n], func=AF.Exp,
                    scale=-0.5, bias=lns_c[:]))

            def emit_winv_chunk(k):
                emit_winv(16 * k, 16 * (k + 1))

            if stage == 3:
                for k in range(NCHUNK):
                    if k < NCHUNK - 1:
                        emit_w_dma(k + 1)
                    for ct in range(16 * k, 16 * (k + 1)):
                        emit_norm(ct)
                    emit_winv_chunk(k)
                probe = sm.tile([1, 1], f32, name="probe")
                nc.vector.tensor_copy(probe[:], winv[0:1, 0:1])
                nc.sync.dma_start(out=out[:], in_=probe[:])
                return nc

            # ---- main loop (norm pipeline runs one chunk ahead) ----
            emit_xhat()
            accs = [sm.tile([128, B], bf16, name=f"acc{a}")
                    for a in range(NACC)]
            sump = spsum.tile([1, B], f32, tag="sp", name="sump")
            sum_state = {"started": False, "ndve": 0}

            for g4 in range(4):
                for ct in range(4 * g4, 4 * g4 + 4):
                    emit_norm(ct)
                emit_winv(4 * g4, 4 * g4 + 4)

            pendq = []  # [(ex_tile, ct)]

            def emit_sum(ex, ct):
                if _sum_on_pe(ct):
                    chain("pe", nc.tensor.matmul(
                        sump[:], lhsT=ones_b[:], rhs=ex[:],
                        start=not sum_state["started"], stop=False,
                        skip_group_check=True))
                    sum_state["started"] = True
                else:
                    a = sum_state["ndve"] % NACC
                    if sum_state["ndve"] < NACC:
                        chain("dve", nc.vector.tensor_copy(accs[a][:], ex[:]))
                    else:
                        chain("dve", nc.vector.tensor_tensor(
                            accs[a][:], accs[a][:], ex[:], OP.add))
                    sum_state["ndve"] += 1

            for k in range(NCHUNK):
                if k < NCHUNK - 2:
                    emit_w_dma(k + 2)
                if k == 1:
                    emit_gather_dma()
                for ct in range(16 * k, 16 * (k + 1)):
                    j, s = ct // 4, ct % 4
                    mp = mpsum.tile([128, B], f32, tag="mp")
                    for g in range(2):
                        chain("pe", nc.tensor.matmul(
                            mp[:],
                            lhsT=WQ[:, j, g, :, 128 * s:128 * (s + 1)],
                            rhs=xq[g][:], start=(g == 0), stop=(g == 1),
                            perf_mode=DR))
                    ex = expp.tile([128, B], bf16, tag="ex")
                    chain("act", nc.scalar.activation(
                        out=ex[:], in_=mp[:], func=AF.Exp,
                        scale=winv[:, ct:ct + 1]))
                    pendq.append((ex, ct))
                    if len(pendq) > 2:
                        emit_sum(*pendq.pop(0))
                    if 32 <= ct < 48 and ct % 4 == 1:
                        emit_gather_piece((ct - 33) // 4)
                    if ct == 46:
                        emit_gather_compute()
                    # next chunk's norm pipeline, spread one-or-two ops
                    # per tile so the DVE queue never blocks on a burst
                    ti = ct % 16
                    if k < NCHUNK - 1 and ti >= 2:
                        base = 16 * (k + 1)
                        if ti <= 13:
                            emit_norm(base + ti - 2)
                        elif ti == 14:
                            emit_norm(base + 12)
                            emit_norm(base + 13)
                        else:
                            emit_norm(base + 14)
                            emit_norm(base + 15)
                            emit_winv_chunk(k + 1)
            for p in pendq:
                emit_sum(*p)

            # ---- fold DVE accumulators into the PE PSUM group ----
            nacc = min(NACC, sum_state["ndve"])
            stride = 1
            while stride < nacc:
                for a in range(0, nacc - stride, 2 * stride):
                    chain("dve", nc.vector.tensor_tensor(
                        accs[a][:], accs[a][:], accs[a + stride][:], OP.add))
                stride *= 2
            chain("pe", nc.tensor.matmul(
                sump[:], lhsT=ones_b[:], rhs=accs[0][:],
                start=not sum_state["started"], stop=True,
                skip_group_check=True))
            sumrow = sm.tile([1, B], f32, name="sumrow")
            chain("act", nc.scalar.activation(out=sumrow[:], in_=sump[:],
                                              func=AF.Copy))
            if stage == 4:
                probe = sm.tile([1, 1], f32, name="probe")
                nc.vector.tensor_copy(probe[:], sumrow[0:1, 0:1])
                nc.sync.dma_start(out=out[:], in_=probe[:])
                return nc

            # ---- single AllReduce: [sums | tgt] (staging on HW DGE) ----
            arin = dpool.tile([1, 2 * B], f32, name="arin")
            arout = dpool.tile([1, 2 * B], f32, name="arout",
                               addr_space="Shared")
            nc.sync.dma_start(out=arin[0:1, 0:B], in_=sumrow[:])
            nc.sync.dma_start(
                out=arin[0:1, B:2 * B].rearrange("a (j p) -> (a p) j", p=128),
                in_=tgt[:])
            chain("gps", nc.gpsimd.collective_compute(
                "AllReduce", OP.add, replica_groups=rg,
                ins=[arin[:]], outs=[arout[:]]))

            fsa = sm.tile([128, 4], f32, name="fsa")
            tg = sm.tile([128, 4], f32, name="tg")
            nc.sync.dma_start(
                out=fsa[:],
                in_=arout[0:1, 0:B].rearrange("a (j p) -> (a p) j", p=128))
            nc.sync.dma_start(
                out=tg[:],
                in_=arout[0:1, B:2 * B].rearrange("a (j p) -> (a p) j",
                                                  p=128))

            # ---- final phase ----
            tcl = sm.tile([128, 4], f32, name="tcl")
            chain("dve", nc.vector.tensor_scalar(
                tcl[:], tg[:], -1.0 + EPS, 1.0 - EPS, OP.max, OP.min))
            om = sm.tile([128, 4], f32, name="om")
            # om = 1 - tcl^2 = (tcl * -1) * tcl + 1... use two ops:
            t2 = sm.tile([128, 4], f32, name="t2")
            chain("dve", nc.vector.tensor_tensor(t2[:], tcl[:], tcl[:],
                                                 OP.mult))
            chain("dve", nc.vector.tensor_scalar(om[:], t2[:], -1.0, 1.0,
                                                 OP.mult, OP.add))
            lnom = sm.tile([128, 4], f32, name="lnom")
            chain("act", nc.scalar.activation(out=lnom[:], in_=om[:],
                                              func=AF.Ln))
            # root' = S*sin(M)*sqrt(om) = exp(0.5*lnom + ln(S*sinM))
            lnsm_c = cpool.tile([128, 1], f32, name="lnsm_c")
            nc.gpsimd.memset(lnsm_c[:], float(math.log(S * math.sin(MARG))))
            rootp = sm.tile([128, 4], f32, name="rootp")
            chain("act", nc.scalar.activation(out=rootp[:], in_=lnom[:],
                                              func=AF.Exp, scale=0.5,
                                              bias=lnsm_c[:]))
            # num = tcl*S*cosM - root'
            num = sm.tile([128, 4], f32, name="num")
            chain("dve", nc.vector.scalar_tensor_tensor(
                out=num[:], in0=tcl[:], scalar=float(S * math.cos(MARG)),
                in1=rootp[:], op0=OP.mult, op1=OP.subtract))
            expnum = sm.tile([128, 4], f32, name="expnum")
            chain("act", nc.scalar.activation(out=expnum[:], in_=num[:],
                                              func=AF.Exp))
            est = sm.tile([128, 4], f32, name="est")
            chain("act", nc.scalar.activation(out=est[:], in_=tg[:],
                                              func=AF.Exp, scale=S))
            # d2 = (fsa - PAD_ONES) - est ; den = d2 + expnum
            d2 = sm.tile([128, 4], f32, name="d2")
            chain("dve", nc.vector.scalar_tensor_tensor(
                out=d2[:], in0=fsa[:], scalar=-PAD_ONES, in1=est[:],
                op0=OP.add, op1=OP.subtract))
            den = sm.tile([128, 4], f32, name="den")
            chain("dve", nc.vector.tensor_tensor(den[:], d2[:], expnum[:],
                                                 OP.add))
            lden = sm.tile([128, 4], f32, name="lden")
            chain("act", nc.scalar.activation(out=lden[:], in_=den[:],
                                              func=AF.Ln))
            # pb = num - lden ; pr = rowsum(pb)
            pbt = sm.tile([128, 4], f32, name="pbt")
            pr = sm.tile([128, 1], f32, name="pr")
            chain("dve", nc.vector.tensor_tensor(pbt[:], num[:], lden[:],
                                                 OP.subtract))
            chain("dve", nc.vector.tensor_reduce(out=pr[:], in_=pbt[:],
                                                 axis=AX.X, op=OP.add))
            fmm = spsum.tile([1, 1], f32, tag="sp2", name="fmm")
            nc.tensor.matmul(fmm[:], lhsT=ones_f[:], rhs=pr[:], start=True,
                             stop=True)
            outsb = sm.tile([1, 1], f32, name="outsb")
            nc.scalar.activation(out=outsb[:], in_=fmm[:], func=AF.Copy,
                                 scale=-1.0 / B)
            nc.sync.dma_start(out=out[:], in_=outsb[:])

    return nc


_CACHE = {}


def make_in_maps(x, y, W):
    import ml_dtypes
    x = np.ascontiguousarray(np.asarray(x, dtype=np.float32))
    y = np.asarray(y).astype(np.int64)
    W = np.asarray(W, dtype=np.float32)
    in_maps = []
    for i in range(NCORES):
        c0 = i * CL
        Wsh = np.ascontiguousarray(W[c0:c0 + CL])           # [CL, D]
        Wp = np.zeros((CPAD, D), dtype=np.float32)
        Wp[:CL] = Wsh * WS
        # wq[p, j, g, i, c] = WS * W^T[g*256+i*128+p, j*512+c]
        wq_i = np.ascontiguousarray(
            Wp.reshape(NJ, 512, 2, 2, 128).transpose(4, 0, 2, 3, 1)
        ).astype(ml_dtypes.float8_e4m3)
        # wn2[p, t, d] = WS * W[t*128+p, d]
        wn2_i = np.ascontiguousarray(
            Wp.reshape(NT, 128, D).transpose(1, 0, 2)
        ).astype(ml_dtypes.bfloat16)
        yloc = np.clip(y - c0, 0, CL - 1).astype(np.int32)  # [B]
        valid = ((y >= c0) & (y < c0 + CL)).astype(np.float32)
        in_maps.append({
            "wq": wq_i,
            "wn2": wn2_i,
            "wn": Wsh,
            "x": x,
            "yi": np.ascontiguousarray(yloc.reshape(4, 128).T),
            "yv": np.ascontiguousarray(valid.reshape(4, 128).T),
        })
    return in_maps


def kernel(x, y, W, _trace=False, _stage=9):
    from concourse.bass_utils import run_bass_kernel_spmd
    key = ("nc", _stage)
    if key not in _CACHE:
        _CACHE[key] = build(_stage)
    in_maps = make_in_maps(x, y, W)
    res = run_bass_kernel_spmd(_CACHE[key], in_maps, list(range(NCORES)),
                               trace=_trace)
    val = np.float32(res.results[0]["out"][0, 0])
    if _trace:
        return val, res
    return val
